# revision 11
# baseline (speedup 1.0000x reference)
"""3-layer GAT (DGL GATConv) on 8 Trainium2 NeuronCores.

Sharding (per hint): nodes partitioned contiguously across 8 cores (6250
each); edges partitioned by dst so segment softmax + scatter-add are
device-local. Halo exchange = per-layer 8-core DRAM AllGather of a bf16
feature table (row = [feat | el | pad], 256B-aligned rows for dma_gather).

Per core, dst nodes form 49 groups of 128. Each group's edges are padded to a
fixed capacity and fetched via SWDGE dma_gather (one lo/hi pair of gathers per
group-pair; the int16 index limit is handled by splitting the table at row
25000). Per 128-edge tile, aggregation is one PE matmul against a host-built
one-hot scatter matrix S^T (fp8, exact); er_dst is expanded edge-wise via the
transposed one-hot S. Softmax skips max-subtraction (scores are O(1); the
result is mathematically identical): p = exp(lrelu(s)) = max(exp(s),
exp(0.2 s)). Numerator and denominator come out of the same matmul (p is
appended as extra rhs columns).

Layer 0's table depends only on inputs, so it is computed on the host and
uploaded — no layer-0 node stage or collective on device.
"""

import numpy as np
import ml_dtypes

import concourse.bacc as bacc
import concourse.mybir as mybir
import concourse.tile as tile
from concourse import library_config
from concourse.bass_utils import run_bass_kernel_spmd
from concourse.masks import make_identity

N = 50000
E = 800000
F_IN = 128
HID = 64
OUT = 40
NEG = 0.2

NCORES = 8
NSH = N // NCORES            # 6250 nodes per core
G = (NSH + 127) // 128       # 49 groups of 128 dst nodes
HALF = N // 2                # table split for int16 gather indices

BF16 = ml_dtypes.bfloat16
FP8 = ml_dtypes.float8_e4m3

_AL = mybir.AluOpType
_AF = mybir.ActivationFunctionType
_dt = mybir.dt


def _wrap_idx(seq):
    """[n] int array -> [128, n/16] int16 gather-index layout
    (idx i at partition i%16, col i//16; replicated to all 8 Q7 cores)."""
    n = len(seq)
    blk = np.asarray(seq, np.int16).reshape(n // 16, 16).T
    return np.tile(blk, (8, 1))


def _pairs():
    prs = [(2 * i, 2 * i + 1) for i in range(G // 2)]
    if G % 2:
        prs.append((G - 1,))
    return prs


def _preprocess(src, dst):
    """Per-core edge partition, padded slot assignment, one-hot matrices."""
    per_core = []
    counts_max = 1
    for c in range(NCORES):
        mask = (dst // NSH) == c
        s = src[mask]
        dl = dst[mask] - c * NSH
        g = dl >> 7
        rel = dl & 127
        lo = s < HALF
        per_core.append((s, g, rel, lo))
        for gg in range(G):
            in_g = g == gg
            counts_max = max(counts_max,
                             int(np.count_nonzero(in_g & lo)),
                             int(np.count_nonzero(in_g & ~lo)))
    cap = ((counts_max + 127) // 128) * 128
    th = cap // 128          # V tiles per half per group
    tpg = 2 * th             # V tiles per group

    cores = []
    for c in range(NCORES):
        s, g, rel, lo = per_core[c]
        idx_cols = []
        st = np.zeros((G, tpg, 128, 128), np.uint8)
        ss = np.zeros((G, 128, tpg, 128), np.uint8)
        for pr in _pairs():
            npg = len(pr)
            for half in (0, 1):
                seq = np.zeros(npg * cap, np.int64)
                for gi, gg in enumerate(pr):
                    m = (g == gg) & (lo if half == 0 else ~lo)
                    es = s[m] - (0 if half == 0 else HALF)
                    rl = rel[m]
                    k = len(es)
                    assert k <= cap
                    seq[gi * cap:gi * cap + k] = es
                    slot = np.arange(k)
                    t_loc = half * th + slot // 128
                    lane = slot % 128
                    st[gg, t_loc, lane, rl] = 1
                    ss[gg, rl, t_loc, lane] = 1
                idx_cols.append(_wrap_idx(seq))
        cores.append(dict(
            idx=np.concatenate(idx_cols, axis=1),
            st=st.astype(FP8),
            ss=ss.astype(FP8).reshape(G, 128, tpg * 128),
        ))
    return cores, cap, th, tpg


def _node_major(arr, c):
    """[N, k] -> [128, G*k] f32 for core c's shard (zero-padded)."""
    k = arr.shape[1]
    out = np.zeros((G * 128, k), np.float32)
    out[:NSH] = arr[c * NSH:(c + 1) * NSH]
    return np.ascontiguousarray(
        out.reshape(G, 128, k).transpose(1, 0, 2).reshape(128, G * k))


def _build_program(cap, th, tpg):
    nc = bacc.Bacc("TRN2", target_bir_lowering=False, debug=False,
                   num_devices=NCORES)
    f32, bf16, fp8, i16 = _dt.float32, _dt.bfloat16, _dt.float8e4, _dt.int16
    IDXC = 2 * G * cap // 16

    table0 = nc.dram_tensor("table0", [N, 256], bf16, kind="ExternalInput")
    x_nd = nc.dram_tensor("x_nd", [128, G * 128], f32, kind="ExternalInput")
    er0_in = nc.dram_tensor("er0_in", [128, G * 2], bf16, kind="ExternalInput")
    idx_in = nc.dram_tensor("idx_in", [128, IDXC], i16, kind="ExternalInput")
    st_in = nc.dram_tensor("st_in", [G, tpg, 128, 128], fp8, kind="ExternalInput")
    ss_in = nc.dram_tensor("ss_in", [G, 128, tpg * 128], fp8, kind="ExternalInput")
    w1_in = nc.dram_tensor("w1_in", [128, 128], f32, kind="ExternalInput")
    wle1_in = nc.dram_tensor("wle1_in", [128, 4], f32, kind="ExternalInput")
    b0_in = nc.dram_tensor("b0_in", [128, 128], f32, kind="ExternalInput")
    b1_in = nc.dram_tensor("b1_in", [128, 128], f32, kind="ExternalInput")
    w2_in = nc.dram_tensor("w2_in", [128, OUT], f32, kind="ExternalInput")
    wle2_in = nc.dram_tensor("wle2_in", [128, 2], f32, kind="ExternalInput")
    b2_in = nc.dram_tensor("b2_in", [128, OUT], f32, kind="ExternalInput")
    out_d = nc.dram_tensor("out_lsm", [NSH, OUT], f32, kind="ExternalOutput")

    with tile.TileContext(nc) as tc:
        nc.gpsimd.load_library(library_config.mlp)
        with (
            tc.tile_pool(name="const", bufs=1) as cp,
            tc.tile_pool(name="state", bufs=1) as sp,
            tc.tile_pool(name="stream", bufs=2) as fp,
            tc.tile_pool(name="small", bufs=3) as mp,
            tc.tile_pool(name="psA", bufs=2, space="PSUM") as pA,
            tc.tile_pool(name="psB", bufs=1, space="PSUM") as pB,
            tc.tile_pool(name="dram", bufs=1, space="DRAM") as dp,
        ):
            def const_tile(shape, dtype, src, tag):
                t = cp.tile(shape, dtype, tag=tag)
                nc.sync.dma_start(t[:], src[:])
                return t

            idx_sb = const_tile([128, IDXC], i16, idx_in, "c_idx")
            w1 = const_tile([128, 128], f32, w1_in, "c_w1")
            wle1 = const_tile([128, 4], f32, wle1_in, "c_wle1")
            b0c = const_tile([128, 128], f32, b0_in, "c_b0")
            b1c = const_tile([128, 128], f32, b1_in, "c_b1")
            w2 = const_tile([128, OUT], f32, w2_in, "c_w2")
            wle2 = const_tile([128, 2], f32, wle2_in, "c_wle2")
            b2c = const_tile([128, OUT], f32, b2_in, "c_b2")
            er0 = const_tile([128, G * 2], bf16, er0_in, "c_er0")
            ident = cp.tile([128, 128], f32)
            make_identity(nc, ident[:])

            h1_nd = sp.tile([128, G * 128], f32, tag="h1nd")
            h2_nd = sp.tile([128, G * 128], f32, tag="h2nd")
            hfm = sp.tile([128, G * 128], f32, tag="hfm")  # reused l1 -> l2
            h3 = sp.tile([128, G * OUT], f32, tag="h3")
            er1 = sp.tile([128, G * 2], bf16, tag="er1")
            er2 = sp.tile([128, G * 1], bf16, tag="er2")

            nc.vector.memset(h3[:], 0.0)
            tsh1 = dp.tile([NSH, 256], bf16)
            tfull1 = dp.tile([N, 256], bf16)
            tsh2 = dp.tile([NSH, 128], bf16)
            tfull2 = dp.tile([N, 128], bf16)

            def edge_stage(layer):
                if layer == 0:
                    table, rowc, nh, fdim = table0, 256, 2, 128
                    er_sb, res, bvec = er0, None, b0c
                elif layer == 1:
                    table, rowc, nh, fdim = tfull1, 256, 2, 128
                    er_sb, res, bvec = er1, h1_nd, b1c
                else:
                    table, rowc, nh, fdim = tfull2, 128, 1, OUT
                    er_sb, res, bvec = er2, None, b2c
                vsc = fdim + nh
                hd = fdim // nh
                idx_off = 0

                import os as _os
                _maxp = int(_os.environ.get("GAT_MAXPAIRS", "9999"))
                for _pi, pr in enumerate(_pairs()):
                    if _pi >= _maxp:
                        break
                    npg = len(pr)
                    g0 = pr[0]
                    nt = npg * tpg

                    st_sb = fp.tile([128, npg * tpg * 128], fp8, tag="st")
                    nc.sync.dma_start(
                        st_sb[:],
                        st_in[g0:g0 + npg].transpose([2, 0, 1, 3]))
                    s_sb = fp.tile([128, npg * tpg * 128], fp8, tag="ss")
                    nc.sync.dma_start(
                        s_sb[:],
                        ss_in[g0:g0 + npg].transpose([1, 0, 2]))

                    v = fp.tile([128, nt, rowc], bf16, tag="v")
                    ncols = npg * cap // 16
                    for half in (0, 1):
                        ii = idx_sb[:, idx_off:idx_off + ncols]
                        idx_off += ncols
                        dst_v = v[:, half * (nt // 2):(half + 1) * (nt // 2), :]
                        src_t = table[0:HALF, :] if half == 0 else table[HALF:N, :]
                        nc.gpsimd.dma_gather(
                            dst_v, src_t, ii, npg * cap, npg * cap, rowc,
                            single_packet=False)

                    def vt(gi, t):
                        if t < th:
                            return gi * th + t
                        return npg * th + gi * th + (t - th)

                    # er_dst expand: one matmul per tile into striped PSUM
                    er_ps = pA.tile([128, nt * nh], f32, space="PSUM", tag="erp")
                    for gi in range(npg):
                        gg = pr[gi]
                        for t in range(tpg):
                            v_i = vt(gi, t)
                            nc.tensor.matmul(
                                out=er_ps[:, v_i * nh:(v_i + 1) * nh],
                                lhsT=s_sb[:, (gi * tpg + t) * 128:
                                          (gi * tpg + t + 1) * 128],
                                rhs=er_sb[:, gg * nh:(gg + 1) * nh],
                                start=True, stop=True)
                    score = mp.tile([128, nt * nh], f32, tag="score")
                    nc.vector.tensor_tensor(
                        out=score[:].rearrange("p (t h) -> p t h", h=nh),
                        in0=er_ps[:].rearrange("p (t h) -> p t h", h=nh),
                        in1=v[:, :, fdim:fdim + nh], op=_AL.add)
                    # p = exp(lrelu(s)) = max(exp(s), exp(NEG*s))
                    pa_t = mp.tile([128, nt * nh], f32, tag="pa")
                    pb_t = mp.tile([128, nt * nh], f32, tag="pb")
                    nc.scalar.activation(pa_t[:], score[:], _AF.Exp)
                    nc.scalar.activation(pb_t[:], score[:], _AF.Exp, scale=NEG)
                    p = mp.tile([128, nt * nh], f32, tag="p")
                    nc.vector.tensor_max(p[:], pa_t[:], pb_t[:])

                    vs = fp.tile([128, nt, vsc], bf16, tag="vs")
                    nc.vector.tensor_tensor(
                        out=vs[:, :, 0:fdim].rearrange("p t (h d) -> p t h d", h=nh),
                        in0=v[:, :, 0:fdim].rearrange("p t (h d) -> p t h d", h=nh),
                        in1=p[:].rearrange("p (t h) -> p t h", h=nh)
                            .unsqueeze(3).to_broadcast([128, nt, nh, hd]),
                        op=_AL.mult)
                    nc.vector.tensor_copy(
                        vs[:, :, fdim:fdim + nh],
                        p[:].rearrange("p (t h) -> p t h", h=nh))

                    for gi in range(npg):
                        gg = pr[gi]
                        acc = pA.tile([128, vsc], f32, space="PSUM", tag="acc")
                        for t in range(tpg):
                            nc.tensor.matmul(
                                out=acc[:],
                                lhsT=st_sb[:, (gi * tpg + t) * 128:
                                           (gi * tpg + t + 1) * 128],
                                rhs=vs[:, vt(gi, t), :],
                                start=(t == 0), stop=(t == tpg - 1))
                        ssb = mp.tile([128, nh], f32, tag="ssb")
                        nc.vector.tensor_scalar(
                            ssb[:], acc[:, fdim:fdim + nh], 1e-30, None, _AL.max)
                        rs = mp.tile([128, nh], f32, tag="rs")
                        nc.vector.reciprocal(rs[:], ssb[:])
                        o = mp.tile([128, fdim], f32, tag="o")
                        for h in range(nh):
                            nc.scalar.activation(
                                o[:, h * hd:(h + 1) * hd],
                                acc[:, h * hd:(h + 1) * hd],
                                _AF.Copy, scale=rs[:, h:h + 1])
                        if layer == 2:
                            nc.vector.tensor_add(
                                h3[:, gg * OUT:(gg + 1) * OUT], o[:], b2c[:])
                            continue
                        xb = mp.tile([128, fdim], f32, tag="xb")
                        nc.vector.tensor_add(xb[:], o[:], bvec[:])
                        # elu(x) = max(x,0) + min(exp(min(x,0)) - 1, 0)
                        t1 = mp.tile([128, fdim], f32, tag="t1")
                        nc.vector.tensor_scalar_min(t1[:], xb[:], 0.0)
                        e1 = mp.tile([128, fdim], f32, tag="e1")
                        nc.scalar.activation(e1[:], t1[:], _AF.Exp)
                        t2 = mp.tile([128, fdim], f32, tag="t2")
                        nc.vector.tensor_scalar(
                            t2[:], e1[:], -1.0, 0.0, _AL.add, _AL.min)
                        t3 = mp.tile([128, fdim], f32, tag="t3")
                        nc.vector.tensor_scalar_max(t3[:], xb[:], 0.0)
                        elu = mp.tile([128, fdim], f32, tag="elu")
                        nc.vector.tensor_add(elu[:], t2[:], t3[:])
                        h_nd = h1_nd if layer == 0 else h2_nd
                        hsl = h_nd[:, gg * 128:(gg + 1) * 128]
                        if layer == 0:
                            xres = fp.tile([128, 128], f32, tag="xres")
                            nc.sync.dma_start(
                                xres[:], x_nd[:, gg * 128:(gg + 1) * 128])
                            nc.vector.tensor_add(hsl, elu[:], xres[:])
                        else:
                            nc.vector.tensor_add(
                                hsl, elu[:], res[:, gg * 128:(gg + 1) * 128])
                        # next layer's node stage, fused
                        tp = pB.tile([128, 128], f32, space="PSUM", tag="tp")
                        nc.tensor.transpose(out=tp[:], in_=hsl, identity=ident[:])
                        fsl = hfm[:, gg * 128:(gg + 1) * 128]
                        nc.scalar.copy(fsl, tp[:])
                        wn = w1 if layer == 0 else w2
                        wlen = wle1 if layer == 0 else wle2
                        fnext = 128 if layer == 0 else OUT
                        nhn = 2 if layer == 0 else 1
                        featp = pB.tile([128, fnext], f32, space="PSUM", tag="featp")
                        nc.tensor.matmul(out=featp[:], lhsT=fsl, rhs=wn[:],
                                         start=True, stop=True)
                        elp = pB.tile([128, 2 * nhn], f32, space="PSUM", tag="elp")
                        nc.tensor.matmul(out=elp[:], lhsT=fsl, rhs=wlen[:],
                                         start=True, stop=True)
                        rowcn = 256 if layer == 0 else 128
                        tt = mp.tile([128, rowcn], bf16, tag="ttile")
                        nc.vector.memset(tt[:, fnext + nhn:rowcn], 0)
                        nc.scalar.copy(tt[:, 0:fnext], featp[:])
                        nc.vector.tensor_copy(
                            tt[:, fnext:fnext + nhn], elp[:, 0:nhn])
                        ern = er1 if layer == 0 else er2
                        nc.vector.tensor_copy(
                            ern[:, gg * nhn:(gg + 1) * nhn], elp[:, nhn:2 * nhn])
                        tshn = tsh1 if layer == 0 else tsh2
                        nrows = min(128, NSH - gg * 128)
                        nc.sync.dma_start(
                            tshn[gg * 128:gg * 128 + nrows, :], tt[:nrows, :])

            import os
            stages = os.environ.get("GAT_STAGES", "012")
            edge_stage(0)
            if "c1" not in os.environ.get("GAT_SKIP", ""):
                nc.gpsimd.collective_compute(
                    "AllGather", _AL.bypass,
                    replica_groups=[list(range(NCORES))],
                    ins=[tsh1[:].opt()], outs=[tfull1[:].opt()])
            if "1" in stages:
                edge_stage(1)
            if "c2" not in os.environ.get("GAT_SKIP", "") and "1" in stages:
                nc.gpsimd.collective_compute(
                    "AllGather", _AL.bypass,
                    replica_groups=[list(range(NCORES))],
                    ins=[tsh2[:].opt()], outs=[tfull2[:].opt()])
            if "2" in stages:
                edge_stage(2)

            # log_softmax over classes: x - ln(sum exp(x))
            ex = sp.tile([128, G * OUT], f32, tag="lsx")
            nc.scalar.activation(ex[:], h3[:], _AF.Exp)
            sm = sp.tile([128, G], f32, tag="lss")
            nc.vector.tensor_reduce(
                sm[:], ex[:].rearrange("p (g c) -> p g c", c=OUT),
                axis=mybir.AxisListType.X, op=_AL.add)
            ls = sp.tile([128, G], f32, tag="lsl")
            nc.scalar.activation(ls[:], sm[:], _AF.Ln)
            lsm = sp.tile([128, G * OUT], f32, tag="lsm")
            nc.vector.tensor_tensor(
                out=lsm[:].rearrange("p (g c) -> p g c", c=OUT),
                in0=h3[:].rearrange("p (g c) -> p g c", c=OUT),
                in1=ls[:].unsqueeze(2).to_broadcast([128, G, OUT]),
                op=_AL.subtract)
            nfull = NSH // 128
            nc.sync.dma_start(
                out_d[0:nfull * 128, :].rearrange("(g p) c -> p g c", p=128),
                lsm[:, 0:nfull * OUT].rearrange("p (g c) -> p g c", c=OUT))
            rem = NSH - nfull * 128
            if rem:
                nc.sync.dma_start(
                    out_d[nfull * 128:NSH, :],
                    lsm[0:rem, nfull * OUT:(nfull + 1) * OUT])

    nc.compile()
    return nc


_PROG_CACHE = {}


def kernel(x, src, dst, W0, al0, ar0, b0, W1, al1, ar1, b1,
           W2, al2, ar2, b2, trace=False):
    x = np.asarray(x, np.float32)
    src = np.asarray(src).astype(np.int64)
    dst = np.asarray(dst).astype(np.int64)
    W0, al0, ar0, b0 = (np.asarray(a, np.float32) for a in (W0, al0, ar0, b0))
    W1, al1, ar1, b1 = (np.asarray(a, np.float32) for a in (W1, al1, ar1, b1))
    W2, al2, ar2, b2 = (np.asarray(a, np.float32) for a in (W2, al2, ar2, b2))

    cores, cap, th, tpg = _preprocess(src, dst)

    # host layer-0 node stage
    feat0 = (x @ W0).reshape(N, 2, HID)
    el0 = np.einsum("nhd,hd->nh", feat0, al0).astype(np.float32)
    er0 = np.einsum("nhd,hd->nh", feat0, ar0).astype(np.float32)
    table0 = np.zeros((N, 256), BF16)
    table0[:, 0:128] = feat0.reshape(N, 128).astype(BF16)
    table0[:, 128:130] = el0.astype(BF16)

    wle1 = np.zeros((128, 4), np.float32)
    for h in range(2):
        wle1[:, h] = W1[:, h * HID:(h + 1) * HID] @ al1[h]
        wle1[:, 2 + h] = W1[:, h * HID:(h + 1) * HID] @ ar1[h]
    wle2 = np.zeros((128, 2), np.float32)
    wle2[:, 0] = W2 @ al2[0]
    wle2[:, 1] = W2 @ ar2[0]

    key = (cap,)
    if key not in _PROG_CACHE:
        _PROG_CACHE[key] = _build_program(cap, th, tpg)
    nc = _PROG_CACHE[key]

    in_maps = []
    for c in range(NCORES):
        cc = cores[c]
        in_maps.append(dict(
            table0=table0,
            x_nd=_node_major(x, c),
            er0_in=_node_major(er0, c).astype(BF16),
            idx_in=cc["idx"],
            st_in=cc["st"],
            ss_in=cc["ss"],
            w1_in=W1,
            wle1_in=wle1,
            b0_in=np.tile(b0[None, :], (128, 1)).astype(np.float32),
            b1_in=np.tile(b1[None, :], (128, 1)).astype(np.float32),
            w2_in=W2,
            wle2_in=wle2,
            b2_in=np.tile(b2[None, :], (128, 1)).astype(np.float32),
        ))
    res = run_bass_kernel_spmd(nc, in_maps, core_ids=list(range(NCORES)),
                               trace=trace)
    out = np.concatenate([res.results[c]["out_lsm"] for c in range(NCORES)],
                         axis=0).astype(np.float32)
    kernel._last_result = res
    return out


# revision 12
# speedup vs baseline: 1.5165x; 1.5165x over previous
"""3-layer GAT (DGL GATConv) on 8 Trainium2 NeuronCores.

Sharding (per hint): nodes partitioned contiguously across 8 cores (6250
each); edges partitioned by dst so segment softmax + scatter-add are
device-local. Halo exchange = per-layer 8-core DRAM AllGather of a bf16
feature table (row = [feat | el | pad], 256B-aligned rows for dma_gather).

Per core, dst nodes form 49 groups of 128. Each group's edges are padded to a
fixed capacity and fetched via SWDGE dma_gather (one lo/hi pair of gathers per
group-pair; the int16 index limit is handled by splitting the table at row
25000). Per 128-edge tile, aggregation is one PE matmul against a host-built
one-hot scatter matrix S^T (fp8, exact); er_dst is expanded edge-wise via the
transposed one-hot S. Softmax skips max-subtraction (scores are O(1); the
result is mathematically identical): p = exp(lrelu(s)) = max(exp(s),
exp(0.2 s)). Numerator and denominator come out of the same matmul (p is
appended as extra rhs columns).

Layer 0's table depends only on inputs, so it is computed on the host and
uploaded — no layer-0 node stage or collective on device.
"""

import numpy as np
import ml_dtypes

import concourse.bacc as bacc
import concourse.mybir as mybir
import concourse.tile as tile
from concourse import library_config
from concourse.bass_utils import run_bass_kernel_spmd
from concourse.masks import make_identity

N = 50000
E = 800000
F_IN = 128
HID = 64
OUT = 40
NEG = 0.2

NCORES = 8
NSH = N // NCORES            # 6250 nodes per core
G = (NSH + 127) // 128       # 49 groups of 128 dst nodes
HALF = N // 2                # table split for int16 gather indices

BF16 = ml_dtypes.bfloat16
FP8 = ml_dtypes.float8_e4m3

_AL = mybir.AluOpType
_AF = mybir.ActivationFunctionType
_dt = mybir.dt


def _wrap_idx(seq):
    """[n] int array -> [128, n/16] int16 gather-index layout
    (idx i at partition i%16, col i//16; replicated to all 8 Q7 cores)."""
    n = len(seq)
    blk = np.asarray(seq, np.int16).reshape(n // 16, 16).T
    return np.tile(blk, (8, 1))


def _pairs():
    prs = [(2 * i, 2 * i + 1) for i in range(G // 2)]
    if G % 2:
        prs.append((G - 1,))
    return prs


def _preprocess(src, dst):
    """Per-core edge partition, padded slot assignment, one-hot matrices."""
    per_core = []
    counts_max = 1
    for c in range(NCORES):
        mask = (dst // NSH) == c
        s = src[mask]
        dl = dst[mask] - c * NSH
        g = dl >> 7
        rel = dl & 127
        lo = s < HALF
        per_core.append((s, g, rel, lo))
        for gg in range(G):
            in_g = g == gg
            counts_max = max(counts_max,
                             int(np.count_nonzero(in_g & lo)),
                             int(np.count_nonzero(in_g & ~lo)))
    cap = ((counts_max + 127) // 128) * 128
    th = cap // 128          # V tiles per half per group
    tpg = 2 * th             # V tiles per group

    cores = []
    for c in range(NCORES):
        s, g, rel, lo = per_core[c]
        idx_cols = []
        st = np.zeros((G, tpg, 128, 128), np.uint8)
        ss = np.zeros((G, 128, tpg, 128), np.uint8)
        for pr in _pairs():
            npg = len(pr)
            for half in (0, 1):
                seq = np.zeros(npg * cap, np.int64)
                for gi, gg in enumerate(pr):
                    m = (g == gg) & (lo if half == 0 else ~lo)
                    es = s[m] - (0 if half == 0 else HALF)
                    rl = rel[m]
                    k = len(es)
                    assert k <= cap
                    seq[gi * cap:gi * cap + k] = es
                    slot = np.arange(k)
                    t_loc = half * th + slot // 128
                    lane = slot % 128
                    st[gg, t_loc, lane, rl] = 1
                    ss[gg, rl, t_loc, lane] = 1
                idx_cols.append(_wrap_idx(seq))
        cores.append(dict(
            idx=np.concatenate(idx_cols, axis=1),
            st=np.ascontiguousarray(st.transpose(2, 0, 1, 3)).astype(FP8),
            ss=np.ascontiguousarray(
                ss.reshape(G, 128, tpg * 128).transpose(1, 0, 2)).astype(FP8),
        ))
    return cores, cap, th, tpg


def _node_major(arr, c):
    """[N, k] -> [128, G*k] f32 for core c's shard (zero-padded)."""
    k = arr.shape[1]
    out = np.zeros((G * 128, k), np.float32)
    out[:NSH] = arr[c * NSH:(c + 1) * NSH]
    return np.ascontiguousarray(
        out.reshape(G, 128, k).transpose(1, 0, 2).reshape(128, G * k))


def _build_program(cap, th, tpg):
    nc = bacc.Bacc("TRN2", target_bir_lowering=False, debug=False,
                   num_devices=NCORES)
    f32, bf16, fp8, i16 = _dt.float32, _dt.bfloat16, _dt.float8e4, _dt.int16
    IDXC = 2 * G * cap // 16

    table0 = nc.dram_tensor("table0", [N, 256], bf16, kind="ExternalInput")
    x_nd = nc.dram_tensor("x_nd", [128, G * 128], f32, kind="ExternalInput")
    er0_in = nc.dram_tensor("er0_in", [128, G * 2], bf16, kind="ExternalInput")
    idx_in = nc.dram_tensor("idx_in", [128, IDXC], i16, kind="ExternalInput")
    st_in = nc.dram_tensor("st_in", [128, G, tpg, 128], fp8, kind="ExternalInput")
    ss_in = nc.dram_tensor("ss_in", [128, G, tpg * 128], fp8, kind="ExternalInput")
    w1_in = nc.dram_tensor("w1_in", [128, 128], f32, kind="ExternalInput")
    wle1_in = nc.dram_tensor("wle1_in", [128, 4], f32, kind="ExternalInput")
    b0_in = nc.dram_tensor("b0_in", [128, 128], f32, kind="ExternalInput")
    b1_in = nc.dram_tensor("b1_in", [128, 128], f32, kind="ExternalInput")
    w2_in = nc.dram_tensor("w2_in", [128, OUT], f32, kind="ExternalInput")
    wle2_in = nc.dram_tensor("wle2_in", [128, 2], f32, kind="ExternalInput")
    b2_in = nc.dram_tensor("b2_in", [128, OUT], f32, kind="ExternalInput")
    out_d = nc.dram_tensor("out_lsm", [NSH, OUT], f32, kind="ExternalOutput")

    with tile.TileContext(nc) as tc:
        nc.gpsimd.load_library(library_config.mlp)
        with (
            tc.tile_pool(name="const", bufs=1) as cp,
            tc.tile_pool(name="state", bufs=1) as sp,
            tc.tile_pool(name="stream", bufs=2) as fp,
            tc.tile_pool(name="small", bufs=3) as mp,
            tc.tile_pool(name="psA", bufs=2, space="PSUM") as pA,
            tc.tile_pool(name="psB", bufs=1, space="PSUM") as pB,
            tc.tile_pool(name="dram", bufs=1, space="DRAM") as dp,
        ):
            def const_tile(shape, dtype, src, tag):
                t = cp.tile(shape, dtype, tag=tag)
                nc.sync.dma_start(t[:], src[:])
                return t

            idx_sb = const_tile([128, IDXC], i16, idx_in, "c_idx")
            w1 = const_tile([128, 128], f32, w1_in, "c_w1")
            wle1 = const_tile([128, 4], f32, wle1_in, "c_wle1")
            b0c = const_tile([128, 128], f32, b0_in, "c_b0")
            b1c = const_tile([128, 128], f32, b1_in, "c_b1")
            w2 = const_tile([128, OUT], f32, w2_in, "c_w2")
            wle2 = const_tile([128, 2], f32, wle2_in, "c_wle2")
            b2c = const_tile([128, OUT], f32, b2_in, "c_b2")
            er0 = const_tile([128, G * 2], bf16, er0_in, "c_er0")
            ident = cp.tile([128, 128], f32)
            make_identity(nc, ident[:])

            h1_nd = sp.tile([128, G * 128], f32, tag="h1nd")
            h2_nd = sp.tile([128, G * 128], f32, tag="h2nd")
            hfm = sp.tile([128, G * 128], f32, tag="hfm")  # reused l1 -> l2
            h3 = sp.tile([128, G * OUT], f32, tag="h3")
            er1 = sp.tile([128, G * 2], bf16, tag="er1")
            er2 = sp.tile([128, G * 1], bf16, tag="er2")

            nc.vector.memset(h3[:], 0.0)
            tsh1 = dp.tile([NSH, 256], bf16)
            tfull1 = dp.tile([N, 256], bf16)
            tsh2 = dp.tile([NSH, 128], bf16)
            tfull2 = dp.tile([N, 128], bf16)

            def edge_stage(layer):
                if layer == 0:
                    table, rowc, nh, fdim = table0, 256, 2, 128
                    er_sb, res, bvec = er0, None, b0c
                elif layer == 1:
                    table, rowc, nh, fdim = tfull1, 256, 2, 128
                    er_sb, res, bvec = er1, h1_nd, b1c
                else:
                    table, rowc, nh, fdim = tfull2, 128, 1, OUT
                    er_sb, res, bvec = er2, None, b2c
                vsc = fdim + nh
                hd = fdim // nh
                idx_off = 0

                import os as _os
                _maxp = int(_os.environ.get("GAT_MAXPAIRS", "9999"))
                for _pi, pr in enumerate(_pairs()):
                    if _pi >= _maxp:
                        break
                    npg = len(pr)
                    g0 = pr[0]
                    nt = npg * tpg

                    st_sb = fp.tile([128, npg * tpg * 128], fp8, tag="st")
                    nc.sync.dma_start(
                        st_sb[:], st_in[:, g0:g0 + npg])
                    s_sb = fp.tile([128, npg * tpg * 128], fp8, tag="ss")
                    nc.sync.dma_start(
                        s_sb[:], ss_in[:, g0:g0 + npg])

                    v = fp.tile([128, nt, rowc], bf16, tag="v")
                    ncols = npg * cap // 16
                    for half in (0, 1):
                        ii = idx_sb[:, idx_off:idx_off + ncols]
                        idx_off += ncols
                        dst_v = v[:, half * (nt // 2):(half + 1) * (nt // 2), :]
                        src_t = table[0:HALF, :] if half == 0 else table[HALF:N, :]
                        nc.gpsimd.dma_gather(
                            dst_v, src_t, ii, npg * cap, npg * cap, rowc,
                            single_packet=False)

                    def vt(gi, t):
                        if t < th:
                            return gi * th + t
                        return npg * th + gi * th + (t - th)

                    # er_dst expand: one matmul per tile into striped PSUM
                    er_ps = pA.tile([128, nt * nh], f32, space="PSUM", tag="erp")
                    for gi in range(npg):
                        gg = pr[gi]
                        for t in range(tpg):
                            v_i = vt(gi, t)
                            nc.tensor.matmul(
                                out=er_ps[:, v_i * nh:(v_i + 1) * nh],
                                lhsT=s_sb[:, (gi * tpg + t) * 128:
                                          (gi * tpg + t + 1) * 128],
                                rhs=er_sb[:, gg * nh:(gg + 1) * nh],
                                start=True, stop=True)
                    score = mp.tile([128, nt * nh], f32, tag="score")
                    nc.vector.tensor_tensor(
                        out=score[:].rearrange("p (t h) -> p t h", h=nh),
                        in0=er_ps[:].rearrange("p (t h) -> p t h", h=nh),
                        in1=v[:, :, fdim:fdim + nh], op=_AL.add)
                    # p = exp(lrelu(s)) = max(exp(s), exp(NEG*s))
                    pa_t = mp.tile([128, nt * nh], f32, tag="pa")
                    pb_t = mp.tile([128, nt * nh], f32, tag="pb")
                    nc.scalar.activation(pa_t[:], score[:], _AF.Exp)
                    nc.scalar.activation(pb_t[:], score[:], _AF.Exp, scale=NEG)
                    p = mp.tile([128, nt * nh], f32, tag="p")
                    nc.vector.tensor_max(p[:], pa_t[:], pb_t[:])

                    vs = fp.tile([128, nt, vsc], bf16, tag="vs")
                    nc.vector.tensor_tensor(
                        out=vs[:, :, 0:fdim].rearrange("p t (h d) -> p t h d", h=nh),
                        in0=v[:, :, 0:fdim].rearrange("p t (h d) -> p t h d", h=nh),
                        in1=p[:].rearrange("p (t h) -> p t h", h=nh)
                            .unsqueeze(3).to_broadcast([128, nt, nh, hd]),
                        op=_AL.mult)
                    nc.vector.tensor_copy(
                        vs[:, :, fdim:fdim + nh],
                        p[:].rearrange("p (t h) -> p t h", h=nh))

                    for gi in range(npg):
                        gg = pr[gi]
                        acc = pA.tile([128, vsc], f32, space="PSUM", tag="acc")
                        for t in range(tpg):
                            nc.tensor.matmul(
                                out=acc[:],
                                lhsT=st_sb[:, (gi * tpg + t) * 128:
                                           (gi * tpg + t + 1) * 128],
                                rhs=vs[:, vt(gi, t), :],
                                start=(t == 0), stop=(t == tpg - 1))
                        ssb = mp.tile([128, nh], f32, tag="ssb")
                        nc.vector.tensor_scalar(
                            ssb[:], acc[:, fdim:fdim + nh], 1e-30, None, _AL.max)
                        rs = mp.tile([128, nh], f32, tag="rs")
                        nc.vector.reciprocal(rs[:], ssb[:])
                        o = mp.tile([128, fdim], f32, tag="o")
                        for h in range(nh):
                            nc.scalar.activation(
                                o[:, h * hd:(h + 1) * hd],
                                acc[:, h * hd:(h + 1) * hd],
                                _AF.Copy, scale=rs[:, h:h + 1])
                        if layer == 2:
                            nc.vector.tensor_add(
                                h3[:, gg * OUT:(gg + 1) * OUT], o[:], b2c[:])
                            continue
                        xb = mp.tile([128, fdim], f32, tag="xb")
                        nc.vector.tensor_add(xb[:], o[:], bvec[:])
                        # elu(x) = max(x,0) + min(exp(min(x,0)) - 1, 0)
                        t1 = mp.tile([128, fdim], f32, tag="t1")
                        nc.vector.tensor_scalar_min(t1[:], xb[:], 0.0)
                        e1 = mp.tile([128, fdim], f32, tag="e1")
                        nc.scalar.activation(e1[:], t1[:], _AF.Exp)
                        t2 = mp.tile([128, fdim], f32, tag="t2")
                        nc.vector.tensor_scalar(
                            t2[:], e1[:], -1.0, 0.0, _AL.add, _AL.min)
                        t3 = mp.tile([128, fdim], f32, tag="t3")
                        nc.vector.tensor_scalar_max(t3[:], xb[:], 0.0)
                        elu = mp.tile([128, fdim], f32, tag="elu")
                        nc.vector.tensor_add(elu[:], t2[:], t3[:])
                        h_nd = h1_nd if layer == 0 else h2_nd
                        hsl = h_nd[:, gg * 128:(gg + 1) * 128]
                        if layer == 0:
                            xres = fp.tile([128, 128], f32, tag="xres")
                            nc.sync.dma_start(
                                xres[:], x_nd[:, gg * 128:(gg + 1) * 128])
                            nc.vector.tensor_add(hsl, elu[:], xres[:])
                        else:
                            nc.vector.tensor_add(
                                hsl, elu[:], res[:, gg * 128:(gg + 1) * 128])
                        # next layer's node stage, fused
                        tp = pB.tile([128, 128], f32, space="PSUM", tag="tp")
                        nc.tensor.transpose(out=tp[:], in_=hsl, identity=ident[:])
                        fsl = hfm[:, gg * 128:(gg + 1) * 128]
                        nc.scalar.copy(fsl, tp[:])
                        wn = w1 if layer == 0 else w2
                        wlen = wle1 if layer == 0 else wle2
                        fnext = 128 if layer == 0 else OUT
                        nhn = 2 if layer == 0 else 1
                        featp = pB.tile([128, fnext], f32, space="PSUM", tag="featp")
                        nc.tensor.matmul(out=featp[:], lhsT=fsl, rhs=wn[:],
                                         start=True, stop=True)
                        elp = pB.tile([128, 2 * nhn], f32, space="PSUM", tag="elp")
                        nc.tensor.matmul(out=elp[:], lhsT=fsl, rhs=wlen[:],
                                         start=True, stop=True)
                        rowcn = 256 if layer == 0 else 128
                        tt = mp.tile([128, rowcn], bf16, tag="ttile")
                        nc.vector.memset(tt[:, fnext + nhn:rowcn], 0)
                        nc.scalar.copy(tt[:, 0:fnext], featp[:])
                        nc.vector.tensor_copy(
                            tt[:, fnext:fnext + nhn], elp[:, 0:nhn])
                        ern = er1 if layer == 0 else er2
                        nc.vector.tensor_copy(
                            ern[:, gg * nhn:(gg + 1) * nhn], elp[:, nhn:2 * nhn])
                        tshn = tsh1 if layer == 0 else tsh2
                        nrows = min(128, NSH - gg * 128)
                        nc.sync.dma_start(
                            tshn[gg * 128:gg * 128 + nrows, :], tt[:nrows, :])

            import os
            stages = os.environ.get("GAT_STAGES", "012")
            edge_stage(0)
            if "c1" not in os.environ.get("GAT_SKIP", ""):
                nc.gpsimd.collective_compute(
                    "AllGather", _AL.bypass,
                    replica_groups=[list(range(NCORES))],
                    ins=[tsh1[:].opt()], outs=[tfull1[:].opt()])
            if "1" in stages:
                edge_stage(1)
            if "c2" not in os.environ.get("GAT_SKIP", "") and "1" in stages:
                nc.gpsimd.collective_compute(
                    "AllGather", _AL.bypass,
                    replica_groups=[list(range(NCORES))],
                    ins=[tsh2[:].opt()], outs=[tfull2[:].opt()])
            if "2" in stages:
                edge_stage(2)

            # log_softmax over classes: x - ln(sum exp(x))
            ex = sp.tile([128, G * OUT], f32, tag="lsx")
            nc.scalar.activation(ex[:], h3[:], _AF.Exp)
            sm = sp.tile([128, G], f32, tag="lss")
            nc.vector.tensor_reduce(
                sm[:], ex[:].rearrange("p (g c) -> p g c", c=OUT),
                axis=mybir.AxisListType.X, op=_AL.add)
            ls = sp.tile([128, G], f32, tag="lsl")
            nc.scalar.activation(ls[:], sm[:], _AF.Ln)
            lsm = sp.tile([128, G * OUT], f32, tag="lsm")
            nc.vector.tensor_tensor(
                out=lsm[:].rearrange("p (g c) -> p g c", c=OUT),
                in0=h3[:].rearrange("p (g c) -> p g c", c=OUT),
                in1=ls[:].unsqueeze(2).to_broadcast([128, G, OUT]),
                op=_AL.subtract)
            nfull = NSH // 128
            nc.sync.dma_start(
                out_d[0:nfull * 128, :].rearrange("(g p) c -> p g c", p=128),
                lsm[:, 0:nfull * OUT].rearrange("p (g c) -> p g c", c=OUT))
            rem = NSH - nfull * 128
            if rem:
                nc.sync.dma_start(
                    out_d[nfull * 128:NSH, :],
                    lsm[0:rem, nfull * OUT:(nfull + 1) * OUT])

    nc.compile()
    return nc


_PROG_CACHE = {}


def kernel(x, src, dst, W0, al0, ar0, b0, W1, al1, ar1, b1,
           W2, al2, ar2, b2, trace=False):
    x = np.asarray(x, np.float32)
    src = np.asarray(src).astype(np.int64)
    dst = np.asarray(dst).astype(np.int64)
    W0, al0, ar0, b0 = (np.asarray(a, np.float32) for a in (W0, al0, ar0, b0))
    W1, al1, ar1, b1 = (np.asarray(a, np.float32) for a in (W1, al1, ar1, b1))
    W2, al2, ar2, b2 = (np.asarray(a, np.float32) for a in (W2, al2, ar2, b2))

    cores, cap, th, tpg = _preprocess(src, dst)

    # host layer-0 node stage
    feat0 = (x @ W0).reshape(N, 2, HID)
    el0 = np.einsum("nhd,hd->nh", feat0, al0).astype(np.float32)
    er0 = np.einsum("nhd,hd->nh", feat0, ar0).astype(np.float32)
    table0 = np.zeros((N, 256), BF16)
    table0[:, 0:128] = feat0.reshape(N, 128).astype(BF16)
    table0[:, 128:130] = el0.astype(BF16)

    wle1 = np.zeros((128, 4), np.float32)
    for h in range(2):
        wle1[:, h] = W1[:, h * HID:(h + 1) * HID] @ al1[h]
        wle1[:, 2 + h] = W1[:, h * HID:(h + 1) * HID] @ ar1[h]
    wle2 = np.zeros((128, 2), np.float32)
    wle2[:, 0] = W2 @ al2[0]
    wle2[:, 1] = W2 @ ar2[0]

    key = (cap,)
    if key not in _PROG_CACHE:
        _PROG_CACHE[key] = _build_program(cap, th, tpg)
    nc = _PROG_CACHE[key]

    in_maps = []
    for c in range(NCORES):
        cc = cores[c]
        in_maps.append(dict(
            table0=table0,
            x_nd=_node_major(x, c),
            er0_in=_node_major(er0, c).astype(BF16),
            idx_in=cc["idx"],
            st_in=cc["st"],
            ss_in=cc["ss"],
            w1_in=W1,
            wle1_in=wle1,
            b0_in=np.tile(b0[None, :], (128, 1)).astype(np.float32),
            b1_in=np.tile(b1[None, :], (128, 1)).astype(np.float32),
            w2_in=W2,
            wle2_in=wle2,
            b2_in=np.tile(b2[None, :], (128, 1)).astype(np.float32),
        ))
    res = run_bass_kernel_spmd(nc, in_maps, core_ids=list(range(NCORES)),
                               trace=trace)
    out = np.concatenate([res.results[c]["out_lsm"] for c in range(NCORES)],
                         axis=0).astype(np.float32)
    kernel._last_result = res
    return out


# revision 21
# speedup vs baseline: 76.4428x; 50.4084x over previous
"""3-layer GAT (DGL GATConv) on 8 Trainium2 NeuronCores.

Sharding (per hint): nodes partitioned contiguously across 8 cores (6250
each); edges partitioned by dst so segment softmax + scatter-add are
device-local. Halo exchange = per-layer 8-core DRAM AllGather of a bf16
feature table (row = [feat | el | pad], 256B-aligned rows for dma_gather).

Per core, dst nodes form 49 groups of 128. Each group's edges are padded to a
fixed capacity and fetched via SWDGE dma_gather (one lo/hi pair of gathers per
group-pair; the int16 index limit is handled by splitting the table at row
25000). Per 128-edge tile, aggregation is one PE matmul against a host-built
one-hot scatter matrix S^T (fp8, exact); er_dst is expanded edge-wise via the
transposed one-hot S. Softmax skips max-subtraction (scores are O(1); the
result is mathematically identical): p = exp(lrelu(s)) = max(exp(s),
exp(0.2 s)). Numerator and denominator come out of the same matmul (p is
appended as extra rhs columns).

Layer 0's table depends only on inputs, so it is computed on the host and
uploaded — no layer-0 node stage or collective on device.
"""

import numpy as np
import ml_dtypes

import concourse.bacc as bacc
import concourse.mybir as mybir
import concourse.tile as tile
from concourse import library_config
from concourse.bass_utils import run_bass_kernel_spmd
from concourse.masks import make_identity

N = 50000
E = 800000
F_IN = 128
HID = 64
OUT = 40
NEG = 0.2

NCORES = 8
NSH = N // NCORES            # 6250 nodes per core
G = (NSH + 127) // 128       # 49 groups of 128 dst nodes
HALF = N // 2                # table split for int16 gather indices

BF16 = ml_dtypes.bfloat16
FP8 = ml_dtypes.float8_e4m3

_AL = mybir.AluOpType
_AF = mybir.ActivationFunctionType
_dt = mybir.dt


def _wrap_idx(seq):
    """[n] int array -> [128, n/16] int16 gather-index layout
    (idx i at partition i%16, col i//16; replicated to all 8 Q7 cores)."""
    n = len(seq)
    blk = np.asarray(seq, np.int16).reshape(n // 16, 16).T
    return np.tile(blk, (8, 1))


def _chunk_bounds():
    gstep = max(1, (G + 3) // 4)
    return sorted({min(k * gstep * 128, NSH) for k in range(4)} | {NSH})


def _new_row():
    """Table row permutation making chunked AllGather outputs contiguous:
    global order = [chunk0 core0..7 | chunk1 core0..7 | ...]."""
    bounds = np.array(_chunk_bounds())
    r = np.arange(NSH)
    k = np.searchsorted(bounds[1:], r, side="right")
    rows_k = bounds[1:] - bounds[:-1]
    base_k = NCORES * bounds[:-1]
    within = r - bounds[k]
    out = np.empty(N, np.int64)
    for c in range(NCORES):
        out[c * NSH + r] = base_k[k] + c * rows_k[k] + within
    return out


def _pairs():
    prs = [(2 * i, 2 * i + 1) for i in range(G // 2)]
    if G % 2:
        prs.append((G - 1,))
    return prs


def _preprocess(src, dst):
    """Per-core edge partition, padded slot assignment, one-hot matrices."""
    new_row = _new_row()
    per_core = []
    counts_max = 1
    for c in range(NCORES):
        mask = (dst // NSH) == c
        s = new_row[src[mask]]
        dl = dst[mask] - c * NSH
        g = dl >> 7
        rel = dl & 127
        lo = s < HALF
        per_core.append((s, g, rel, lo))
        for gg in range(G):
            in_g = g == gg
            counts_max = max(counts_max,
                             int(np.count_nonzero(in_g & lo)),
                             int(np.count_nonzero(in_g & ~lo)))
    cap = ((counts_max + 127) // 128) * 128
    th = cap // 128          # V tiles per half per group
    tpg = 2 * th             # V tiles per group

    cores = []
    for c in range(NCORES):
        s, g, rel, lo = per_core[c]
        idx_cols = []
        st = np.zeros((G, tpg, 128, 128), np.uint8)
        ss = np.zeros((G, 128, tpg, 128), np.uint8)
        for pr in _pairs():
            npg = len(pr)
            for half in (0, 1):
                seq = np.zeros(npg * cap, np.int64)
                for gi, gg in enumerate(pr):
                    m = (g == gg) & (lo if half == 0 else ~lo)
                    es = s[m] - (0 if half == 0 else HALF)
                    rl = rel[m]
                    k = len(es)
                    assert k <= cap
                    seq[gi * cap:gi * cap + k] = es
                    slot = np.arange(k)
                    t_loc = half * th + slot // 128
                    lane = slot % 128
                    st[gg, t_loc, lane, rl] = 1
                    ss[gg, rl, t_loc, lane] = 1
                idx_cols.append(_wrap_idx(seq))
        cores.append(dict(
            idx=np.concatenate(idx_cols, axis=1),
            st=np.ascontiguousarray(st.transpose(2, 0, 1, 3)).astype(FP8),
            ss=np.ascontiguousarray(
                ss.reshape(G, 128, tpg * 128).transpose(1, 0, 2)).astype(FP8),
        ))
    return cores, cap, th, tpg


def _node_major(arr, c):
    """[N, k] -> [128, G*k] f32 for core c's shard (zero-padded)."""
    k = arr.shape[1]
    out = np.zeros((G * 128, k), np.float32)
    out[:NSH] = arr[c * NSH:(c + 1) * NSH]
    return np.ascontiguousarray(
        out.reshape(G, 128, k).transpose(1, 0, 2).reshape(128, G * k))


def _build_program(cap, th, tpg, skip_collectives=False):
    nc = bacc.Bacc("TRN2", target_bir_lowering=False, debug=False,
                   num_devices=NCORES)
    f32, bf16, fp8, i16 = _dt.float32, _dt.bfloat16, _dt.float8e4, _dt.int16
    IDXC = 2 * G * cap // 16

    table0 = nc.dram_tensor("table0", [N, 256], bf16, kind="ExternalInput")
    x_nd = nc.dram_tensor("x_nd", [128, G * 128], bf16, kind="ExternalInput")
    er0_in = nc.dram_tensor("er0_in", [128, G * 2], bf16, kind="ExternalInput")
    idx_in = nc.dram_tensor("idx_in", [128, IDXC], i16, kind="ExternalInput")
    st_in = nc.dram_tensor("st_in", [128, G, tpg, 128], fp8, kind="ExternalInput")
    ss_in = nc.dram_tensor("ss_in", [128, G, tpg * 128], fp8, kind="ExternalInput")
    w1_in = nc.dram_tensor("w1_in", [128, 128], bf16, kind="ExternalInput")
    wle1_in = nc.dram_tensor("wle1_in", [128, 4], bf16, kind="ExternalInput")
    b0_in = nc.dram_tensor("b0_in", [128, 128], f32, kind="ExternalInput")
    b1_in = nc.dram_tensor("b1_in", [128, 128], f32, kind="ExternalInput")
    w2_in = nc.dram_tensor("w2_in", [128, OUT], bf16, kind="ExternalInput")
    wle2_in = nc.dram_tensor("wle2_in", [128, 2], bf16, kind="ExternalInput")
    b2_in = nc.dram_tensor("b2_in", [128, OUT], f32, kind="ExternalInput")
    out_d = nc.dram_tensor("out_lsm", [NSH, OUT], f32, kind="ExternalOutput")

    with tile.TileContext(nc) as tc:
        nc.gpsimd.load_library(library_config.mlp)
        with (
            tc.tile_pool(name="const", bufs=1) as cp,
            tc.tile_pool(name="state", bufs=1) as sp,
            tc.tile_pool(name="stream", bufs=3) as fp,
            tc.tile_pool(name="small", bufs=3) as mp,
            tc.tile_pool(name="psA", bufs=2, space="PSUM") as pA,
            tc.tile_pool(name="psB", bufs=1, space="PSUM") as pB,
            tc.tile_pool(name="dram", bufs=1, space="DRAM") as dp,
        ):
            def const_tile(shape, dtype, src, tag):
                t = cp.tile(shape, dtype, tag=tag)
                nc.sync.dma_start(t[:], src[:])
                return t

            idx_sb = const_tile([128, IDXC], i16, idx_in, "c_idx")
            w1 = const_tile([128, 128], bf16, w1_in, "c_w1")
            wle1 = const_tile([128, 4], bf16, wle1_in, "c_wle1")
            b0c = const_tile([128, 128], f32, b0_in, "c_b0")
            b1c = const_tile([128, 128], f32, b1_in, "c_b1")
            w2 = const_tile([128, OUT], bf16, w2_in, "c_w2")
            wle2 = const_tile([128, 2], bf16, wle2_in, "c_wle2")
            b2c = const_tile([128, OUT], f32, b2_in, "c_b2")
            er0 = const_tile([128, G * 2], bf16, er0_in, "c_er0")
            ident = cp.tile([128, 128], bf16, tag="c_ident")
            make_identity(nc, ident[:])

            h1_nd = sp.tile([128, G * 128], bf16, tag="h1nd")
            h2_nd = sp.tile([128, G * 128], bf16, tag="h2nd")
            hfm = sp.tile([128, G * 128], bf16, tag="hfm")  # reused l1 -> l2
            h3 = sp.tile([128, G * OUT], f32, tag="h3")
            er1 = sp.tile([128, G * 2], bf16, tag="er1")
            er2 = sp.tile([128, G * 1], bf16, tag="er2")

            nc.vector.memset(h3[:], 0.0)
            tsh1 = dp.tile([NSH, 256], bf16)
            tfull1 = dp.tile([N, 256], bf16)
            tsh2 = dp.tile([NSH, 128], bf16)
            tfull2 = dp.tile([N, 128], bf16)

            def edge_stage(layer):
                if layer == 0:
                    table, rowc, nh, fdim = table0, 256, 2, 128
                    er_sb, res, bvec = er0, None, b0c
                elif layer == 1:
                    table, rowc, nh, fdim = tfull1, 256, 2, 128
                    er_sb, res, bvec = er1, h1_nd, b1c
                else:
                    table, rowc, nh, fdim = tfull2, 128, 1, OUT
                    er_sb, res, bvec = er2, None, b2c
                vsc = fdim + nh
                hd = fdim // nh
                idx_off = 0

                for pr in _pairs():
                    npg = len(pr)
                    g0 = pr[0]
                    nt = npg * tpg

                    st_sb = fp.tile([128, npg * tpg * 128], fp8, tag="st")
                    nc.sync.dma_start(
                        st_sb[:], st_in[:, g0:g0 + npg])
                    s_sb = fp.tile([128, npg * tpg * 128], fp8, tag="ss")
                    nc.sync.dma_start(
                        s_sb[:], ss_in[:, g0:g0 + npg])

                    v = fp.tile([128, nt, rowc], bf16, tag="v")
                    ncols = npg * cap // 16
                    for half in (0, 1):
                        ii = idx_sb[:, idx_off:idx_off + ncols]
                        idx_off += ncols
                        dst_v = v[:, half * (nt // 2):(half + 1) * (nt // 2), :]
                        src_t = table[0:HALF, :] if half == 0 else table[HALF:N, :]
                        nc.gpsimd.dma_gather(
                            dst_v, src_t, ii, npg * cap, npg * cap, rowc,
                            single_packet=False)

                    def vt(gi, t):
                        if t < th:
                            return gi * th + t
                        return npg * th + gi * th + (t - th)

                    # er_dst expand: one matmul per tile into striped PSUM
                    er_ps = pA.tile([128, nt * nh], f32, space="PSUM", tag="erp")
                    for gi in range(npg):
                        gg = pr[gi]
                        for t in range(tpg):
                            v_i = vt(gi, t)
                            nc.tensor.matmul(
                                out=er_ps[:, v_i * nh:(v_i + 1) * nh],
                                lhsT=s_sb[:, (gi * tpg + t) * 128:
                                          (gi * tpg + t + 1) * 128],
                                rhs=er_sb[:, gg * nh:(gg + 1) * nh],
                                start=True, stop=True)
                    score = mp.tile([128, nt * nh], f32, tag="score")
                    nc.vector.tensor_tensor(
                        out=score[:].rearrange("p (t h) -> p t h", h=nh),
                        in0=er_ps[:].rearrange("p (t h) -> p t h", h=nh),
                        in1=v[:, :, fdim:fdim + nh], op=_AL.add)
                    # p = exp(lrelu(s)) = max(exp(s), exp(NEG*s))
                    pa_t = mp.tile([128, nt * nh], f32, tag="pa")
                    pb_t = mp.tile([128, nt * nh], f32, tag="pb")
                    nc.scalar.activation(pa_t[:], score[:], _AF.Exp)
                    nc.scalar.activation(pb_t[:], score[:], _AF.Exp, scale=NEG)
                    p = mp.tile([128, nt * nh], bf16, tag="p")
                    nc.vector.tensor_max(p[:], pa_t[:], pb_t[:])

                    vs = fp.tile([128, nt, vsc], bf16, tag="vs")
                    nc.vector.tensor_tensor(
                        out=vs[:, :, 0:fdim].rearrange("p t (d h) -> p t d h", h=nh),
                        in0=v[:, :, 0:fdim].rearrange("p t (d h) -> p t d h", h=nh),
                        in1=p[:].rearrange("p (t h) -> p t h", h=nh)
                            .unsqueeze(2).to_broadcast([128, nt, hd, nh]),
                        op=_AL.mult)
                    nc.vector.tensor_copy(
                        vs[:, :, fdim:fdim + nh],
                        p[:].rearrange("p (t h) -> p t h", h=nh))

                    for gi in range(npg):
                        gg = pr[gi]
                        acc = pA.tile([128, vsc], f32, space="PSUM", tag="acc")
                        for t in range(tpg):
                            nc.tensor.matmul(
                                out=acc[:],
                                lhsT=st_sb[:, (gi * tpg + t) * 128:
                                           (gi * tpg + t + 1) * 128],
                                rhs=vs[:, vt(gi, t), :],
                                start=(t == 0), stop=(t == tpg - 1))
                        ssb = mp.tile([128, nh], f32, tag="ssb")
                        nc.vector.tensor_scalar(
                            ssb[:], acc[:, fdim:fdim + nh], 1e-30, None, _AL.max)
                        rs = mp.tile([128, nh], f32, tag="rs")
                        nc.vector.reciprocal(rs[:], ssb[:])
                        o = mp.tile([128, fdim], f32, tag="o")
                        ov = o[:].rearrange("p (d h) -> p d h", h=nh)
                        av = acc[:, 0:fdim].rearrange("p (d h) -> p d h", h=nh)
                        for h in range(nh):
                            nc.scalar.activation(
                                ov[:, :, h:h + 1], av[:, :, h:h + 1],
                                _AF.Copy, scale=rs[:, h:h + 1])
                        if layer == 2:
                            nc.vector.tensor_add(
                                h3[:, gg * OUT:(gg + 1) * OUT], o[:], b2c[:])
                            continue
                        xb = mp.tile([128, fdim], f32, tag="xb")
                        nc.vector.tensor_add(xb[:], o[:], bvec[:])
                        # elu(x) = max(x,0) + min(exp(min(x,0)) - 1, 0)
                        t1 = mp.tile([128, fdim], f32, tag="t1")
                        nc.vector.tensor_scalar_min(t1[:], xb[:], 0.0)
                        e1 = mp.tile([128, fdim], f32, tag="e1")
                        nc.scalar.activation(e1[:], t1[:], _AF.Exp)
                        t2 = mp.tile([128, fdim], f32, tag="t2")
                        nc.vector.tensor_scalar(
                            t2[:], e1[:], -1.0, 0.0, _AL.add, _AL.min)
                        t3 = mp.tile([128, fdim], f32, tag="t3")
                        nc.vector.tensor_scalar_max(t3[:], xb[:], 0.0)
                        elu = mp.tile([128, fdim], f32, tag="elu")
                        nc.vector.tensor_add(elu[:], t2[:], t3[:])
                        h_nd = h1_nd if layer == 0 else h2_nd
                        hsl = h_nd[:, gg * 128:(gg + 1) * 128]
                        if layer == 0:
                            xres = fp.tile([128, 128], bf16, tag="xres")
                            nc.sync.dma_start(
                                xres[:], x_nd[:, gg * 128:(gg + 1) * 128])
                            nc.vector.tensor_add(hsl, elu[:], xres[:])
                        else:
                            nc.vector.tensor_add(
                                hsl, elu[:], res[:, gg * 128:(gg + 1) * 128])
                        # next layer's node stage, fused
                        tp = pB.tile([128, 128], bf16, space="PSUM", tag="tp")
                        nc.tensor.transpose(out=tp[:], in_=hsl, identity=ident[:])
                        fsl = hfm[:, gg * 128:(gg + 1) * 128]
                        nc.scalar.copy(fsl, tp[:])
                        wn = w1 if layer == 0 else w2
                        wlen = wle1 if layer == 0 else wle2
                        fnext = 128 if layer == 0 else OUT
                        nhn = 2 if layer == 0 else 1
                        featp = pB.tile([128, fnext], f32, space="PSUM", tag="featp")
                        nc.tensor.matmul(out=featp[:], lhsT=fsl, rhs=wn[:],
                                         start=True, stop=True)
                        elp = pB.tile([128, 2 * nhn], f32, space="PSUM", tag="elp")
                        nc.tensor.matmul(out=elp[:], lhsT=fsl, rhs=wlen[:],
                                         start=True, stop=True)
                        rowcn = 256 if layer == 0 else 128
                        tt = mp.tile([128, rowcn], bf16, tag="ttile")
                        nc.vector.memset(tt[:, fnext + nhn:rowcn], 0)
                        nc.scalar.copy(tt[:, 0:fnext], featp[:])
                        nc.vector.tensor_copy(
                            tt[:, fnext:fnext + nhn], elp[:, 0:nhn])
                        ern = er1 if layer == 0 else er2
                        nc.vector.tensor_copy(
                            ern[:, gg * nhn:(gg + 1) * nhn], elp[:, nhn:2 * nhn])
                        tshn = tsh1 if layer == 0 else tsh2
                        nrows = min(128, NSH - gg * 128)
                        nc.sync.dma_start(
                            tshn[gg * 128:gg * 128 + nrows, :], tt[:nrows, :])

            def chunked_allgather(tsh, tfull):
                # per-chunk collectives overlap halo exchange with the
                # producing layer's tail groups; table rows are permuted on
                # the host (_new_row) so each chunk's output is contiguous
                bounds = _chunk_bounds()
                for lo, hi in zip(bounds[:-1], bounds[1:]):
                    nc.gpsimd.collective_compute(
                        "AllGather", _AL.bypass,
                        replica_groups=[list(range(NCORES))],
                        ins=[tsh[lo:hi, :].opt()],
                        outs=[tfull[NCORES * lo:NCORES * hi, :].opt()])

            edge_stage(0)
            if not skip_collectives:
                chunked_allgather(tsh1, tfull1)
            edge_stage(1)
            if not skip_collectives:
                chunked_allgather(tsh2, tfull2)
            edge_stage(2)

            # log_softmax over classes: x - ln(sum exp(x))
            ex = sp.tile([128, G * OUT], f32, tag="lsx")
            nc.scalar.activation(ex[:], h3[:], _AF.Exp)
            sm = sp.tile([128, G], f32, tag="lss")
            nc.vector.tensor_reduce(
                sm[:], ex[:].rearrange("p (g c) -> p g c", c=OUT),
                axis=mybir.AxisListType.X, op=_AL.add)
            ls = sp.tile([128, G], f32, tag="lsl")
            nc.scalar.activation(ls[:], sm[:], _AF.Ln)
            lsm = sp.tile([128, G * OUT], f32, tag="lsm")
            nc.vector.tensor_tensor(
                out=lsm[:].rearrange("p (g c) -> p g c", c=OUT),
                in0=h3[:].rearrange("p (g c) -> p g c", c=OUT),
                in1=ls[:].unsqueeze(2).to_broadcast([128, G, OUT]),
                op=_AL.subtract)
            nfull = NSH // 128
            nc.sync.dma_start(
                out_d[0:nfull * 128, :].rearrange("(g p) c -> p g c", p=128),
                lsm[:, 0:nfull * OUT].rearrange("p (g c) -> p g c", c=OUT))
            rem = NSH - nfull * 128
            if rem:
                nc.sync.dma_start(
                    out_d[nfull * 128:NSH, :],
                    lsm[0:rem, nfull * OUT:(nfull + 1) * OUT])

    nc.compile()
    return nc


_PROG_CACHE = {}


def kernel(x, src, dst, W0, al0, ar0, b0, W1, al1, ar1, b1,
           W2, al2, ar2, b2, trace=False):
    x = np.asarray(x, np.float32)
    src = np.asarray(src).astype(np.int64)
    dst = np.asarray(dst).astype(np.int64)
    W0, al0, ar0, b0 = (np.asarray(a, np.float32) for a in (W0, al0, ar0, b0))
    W1, al1, ar1, b1 = (np.asarray(a, np.float32) for a in (W1, al1, ar1, b1))
    W2, al2, ar2, b2 = (np.asarray(a, np.float32) for a in (W2, al2, ar2, b2))

    cores, cap, th, tpg = _preprocess(src, dst)

    # head-interleaved feature order: new col j=(d,h) <- orig col h*64+d.
    # Makes the per-edge attention scale broadcast innermost-stride-1 on DVE.
    PERM = np.array([(j % 2) * HID + j // 2 for j in range(128)])

    # host layer-0 node stage
    feat0 = (x @ W0).reshape(N, 2, HID)
    el0 = np.einsum("nhd,hd->nh", feat0, al0).astype(np.float32)
    er0 = np.einsum("nhd,hd->nh", feat0, ar0).astype(np.float32)
    table0 = np.zeros((N, 256), BF16)
    nr = _new_row()
    table0[nr, 0:128] = feat0.reshape(N, 128)[:, PERM].astype(BF16)
    table0[nr, 128:130] = el0.astype(BF16)

    W1p = W1[PERM, :]              # rows: h1 arrives interleaved
    wle1 = np.zeros((128, 4), np.float32)
    for h in range(2):
        wle1[:, h] = W1p[:, h * HID:(h + 1) * HID] @ al1[h]
        wle1[:, 2 + h] = W1p[:, h * HID:(h + 1) * HID] @ ar1[h]
    W1pi = W1p[:, PERM]            # cols: feat1 comes out interleaved
    W2p = W2[PERM, :]
    wle2 = np.zeros((128, 2), np.float32)
    wle2[:, 0] = W2p @ al2[0]
    wle2[:, 1] = W2p @ ar2[0]

    key = (cap,)
    if key not in _PROG_CACHE:
        _PROG_CACHE[key] = _build_program(cap, th, tpg)
    nc = _PROG_CACHE[key]

    in_maps = []
    for c in range(NCORES):
        cc = cores[c]
        in_maps.append(dict(
            table0=table0,
            x_nd=_node_major(x[:, PERM], c).astype(BF16),
            er0_in=_node_major(er0, c).astype(BF16),
            idx_in=cc["idx"],
            st_in=cc["st"],
            ss_in=cc["ss"],
            w1_in=W1pi.astype(BF16),
            wle1_in=wle1.astype(BF16),
            b0_in=np.tile(b0[None, PERM], (128, 1)).astype(np.float32),
            b1_in=np.tile(b1[None, PERM], (128, 1)).astype(np.float32),
            w2_in=W2p.astype(BF16),
            wle2_in=wle2.astype(BF16),
            b2_in=np.tile(b2[None, :], (128, 1)).astype(np.float32),
        ))
    res = run_bass_kernel_spmd(nc, in_maps, core_ids=list(range(NCORES)),
                               trace=trace)
    out = np.concatenate([res.results[c]["out_lsm"] for c in range(NCORES)],
                         axis=0).astype(np.float32)
    kernel._last_result = res
    return out


# revision 25
# speedup vs baseline: 76.8637x; 1.0055x over previous
"""3-layer GAT (DGL GATConv) on 8 Trainium2 NeuronCores.

Sharding (per hint): nodes partitioned contiguously across 8 cores (6250
each); edges partitioned by dst so segment softmax + scatter-add are
device-local. Halo exchange = per-layer 8-core DRAM AllGather of a bf16
feature table (row = [feat | el | pad], 256B-aligned rows for dma_gather).

Per core, dst nodes form 49 groups of 128. Each group's edges are padded to a
fixed capacity and fetched via SWDGE dma_gather (one lo/hi pair of gathers per
group-pair; the int16 index limit is handled by splitting the table at row
25000). Per 128-edge tile, aggregation is one PE matmul against a host-built
one-hot scatter matrix S^T (fp8, exact); er_dst is expanded edge-wise via the
transposed one-hot S. Softmax skips max-subtraction (scores are O(1); the
result is mathematically identical): p = exp(lrelu(s)) = max(exp(s),
exp(0.2 s)). Numerator and denominator come out of the same matmul (p is
appended as extra rhs columns).

Layer 0's table depends only on inputs, so it is computed on the host and
uploaded — no layer-0 node stage or collective on device.
"""

import numpy as np
import ml_dtypes

import concourse.bacc as bacc
import concourse.mybir as mybir
import concourse.tile as tile
from concourse import library_config
from concourse.bass_utils import run_bass_kernel_spmd
from concourse.masks import make_identity

N = 50000
E = 800000
F_IN = 128
HID = 64
OUT = 40
NEG = 0.2

NCORES = 8
NSH = N // NCORES            # 6250 nodes per core
G = (NSH + 127) // 128       # 49 groups of 128 dst nodes
HALF = N // 2                # table split for int16 gather indices

BF16 = ml_dtypes.bfloat16
FP8 = ml_dtypes.float8_e4m3

_AL = mybir.AluOpType
_AF = mybir.ActivationFunctionType
_dt = mybir.dt


def _wrap_idx(seq):
    """[n] int array -> [128, n/16] int16 gather-index layout
    (idx i at partition i%16, col i//16; replicated to all 8 Q7 cores)."""
    n = len(seq)
    blk = np.asarray(seq, np.int16).reshape(n // 16, 16).T
    return np.tile(blk, (8, 1))


def _chunk_bounds():
    gstep = max(1, (G + 3) // 4)
    return sorted({min(k * gstep * 128, NSH) for k in range(4)} | {NSH})


def _new_row():
    """Table row permutation making chunked AllGather outputs contiguous:
    global order = [chunk0 core0..7 | chunk1 core0..7 | ...]."""
    bounds = np.array(_chunk_bounds())
    r = np.arange(NSH)
    k = np.searchsorted(bounds[1:], r, side="right")
    rows_k = bounds[1:] - bounds[:-1]
    base_k = NCORES * bounds[:-1]
    within = r - bounds[k]
    out = np.empty(N, np.int64)
    for c in range(NCORES):
        out[c * NSH + r] = base_k[k] + c * rows_k[k] + within
    return out


def _pairs():
    prs = [(2 * i, 2 * i + 1) for i in range(G // 2)]
    if G % 2:
        prs.append((G - 1,))
    return prs


def _preprocess(src, dst):
    """Per-core edge partition, padded slot assignment, one-hot matrices."""
    new_row = _new_row()
    per_core = []
    counts_max = 1
    for c in range(NCORES):
        mask = (dst // NSH) == c
        s = new_row[src[mask]]
        dl = dst[mask] - c * NSH
        g = dl >> 7
        rel = dl & 127
        lo = s < HALF
        per_core.append((s, g, rel, lo))
        for gg in range(G):
            in_g = g == gg
            counts_max = max(counts_max,
                             int(np.count_nonzero(in_g & lo)),
                             int(np.count_nonzero(in_g & ~lo)))
    cap = ((counts_max + 127) // 128) * 128
    th = cap // 128          # V tiles per half per group
    tpg = 2 * th             # V tiles per group

    cores = []
    for c in range(NCORES):
        s, g, rel, lo = per_core[c]
        idx_cols = []
        st = np.zeros((G, tpg, 128, 128), np.uint8)
        ss = np.zeros((G, 128, tpg, 128), np.uint8)
        for pr in _pairs():
            npg = len(pr)
            for half in (0, 1):
                seq = np.zeros(npg * cap, np.int64)
                for gi, gg in enumerate(pr):
                    m = (g == gg) & (lo if half == 0 else ~lo)
                    es = s[m] - (0 if half == 0 else HALF)
                    rl = rel[m]
                    k = len(es)
                    assert k <= cap
                    seq[gi * cap:gi * cap + k] = es
                    slot = np.arange(k)
                    t_loc = half * th + slot // 128
                    lane = slot % 128
                    st[gg, t_loc, lane, rl] = 1
                    ss[gg, rl, t_loc, lane] = 1
                idx_cols.append(_wrap_idx(seq))
        cores.append(dict(
            idx=np.concatenate(idx_cols, axis=1),
            st=np.ascontiguousarray(st.transpose(2, 0, 1, 3)).astype(FP8),
            ss=np.ascontiguousarray(
                ss.reshape(G, 128, tpg * 128).transpose(1, 0, 2)).astype(FP8),
        ))
    return cores, cap, th, tpg


def _node_major(arr, c):
    """[N, k] -> [128, G*k] f32 for core c's shard (zero-padded)."""
    k = arr.shape[1]
    out = np.zeros((G * 128, k), np.float32)
    out[:NSH] = arr[c * NSH:(c + 1) * NSH]
    return np.ascontiguousarray(
        out.reshape(G, 128, k).transpose(1, 0, 2).reshape(128, G * k))


def _build_program(cap, th, tpg, skip_collectives=False):
    nc = bacc.Bacc("TRN2", target_bir_lowering=False, debug=False,
                   num_devices=NCORES)
    f32, bf16, fp8, i16 = _dt.float32, _dt.bfloat16, _dt.float8e4, _dt.int16
    IDXC = 2 * G * cap // 16

    table0 = nc.dram_tensor("table0", [N, 256], bf16, kind="ExternalInput")
    x_nd = nc.dram_tensor("x_nd", [128, G * 128], bf16, kind="ExternalInput")
    er0_in = nc.dram_tensor("er0_in", [128, G * 2], bf16, kind="ExternalInput")
    idx_in = nc.dram_tensor("idx_in", [128, IDXC], i16, kind="ExternalInput")
    st_in = nc.dram_tensor("st_in", [128, G, tpg, 128], fp8, kind="ExternalInput")
    ss_in = nc.dram_tensor("ss_in", [128, G, tpg * 128], fp8, kind="ExternalInput")
    w1_in = nc.dram_tensor("w1_in", [128, 128], bf16, kind="ExternalInput")
    wle1_in = nc.dram_tensor("wle1_in", [128, 4], bf16, kind="ExternalInput")
    b0_in = nc.dram_tensor("b0_in", [128, 128], f32, kind="ExternalInput")
    b1_in = nc.dram_tensor("b1_in", [128, 128], f32, kind="ExternalInput")
    w2_in = nc.dram_tensor("w2_in", [128, OUT], bf16, kind="ExternalInput")
    wle2_in = nc.dram_tensor("wle2_in", [128, 2], bf16, kind="ExternalInput")
    b2_in = nc.dram_tensor("b2_in", [128, OUT], f32, kind="ExternalInput")
    out_d = nc.dram_tensor("out_lsm", [NSH, OUT], f32, kind="ExternalOutput")

    with tile.TileContext(nc) as tc:
        nc.gpsimd.load_library(library_config.mlp)
        with (
            tc.tile_pool(name="const", bufs=1) as cp,
            tc.tile_pool(name="state", bufs=1) as sp,
            tc.tile_pool(name="stream", bufs=3) as fp,
            tc.tile_pool(name="small", bufs=3) as mp,
            tc.tile_pool(name="psA", bufs=2, space="PSUM") as pA,
            tc.tile_pool(name="psC", bufs=3, space="PSUM") as pC,
            tc.tile_pool(name="psB", bufs=1, space="PSUM") as pB,
            tc.tile_pool(name="dram", bufs=1, space="DRAM") as dp,
        ):
            def const_tile(shape, dtype, src, tag):
                t = cp.tile(shape, dtype, tag=tag)
                nc.sync.dma_start(t[:], src[:])
                return t

            idx_sb = const_tile([128, IDXC], i16, idx_in, "c_idx")
            w1 = const_tile([128, 128], bf16, w1_in, "c_w1")
            wle1 = const_tile([128, 4], bf16, wle1_in, "c_wle1")
            b0c = const_tile([128, 128], f32, b0_in, "c_b0")
            b1c = const_tile([128, 128], f32, b1_in, "c_b1")
            w2 = const_tile([128, OUT], bf16, w2_in, "c_w2")
            wle2 = const_tile([128, 2], bf16, wle2_in, "c_wle2")
            b2c = const_tile([128, OUT], f32, b2_in, "c_b2")
            er0 = const_tile([128, G * 2], bf16, er0_in, "c_er0")
            ident = cp.tile([128, 128], bf16, tag="c_ident")
            make_identity(nc, ident[:])

            h1_nd = sp.tile([128, G * 128], bf16, tag="h1nd")
            h2_nd = sp.tile([128, G * 128], bf16, tag="h2nd")
            hfm = sp.tile([128, G * 128], bf16, tag="hfm")  # reused l1 -> l2
            h3 = sp.tile([128, G * OUT], f32, tag="h3")
            er1 = sp.tile([128, G * 2], bf16, tag="er1")
            er2 = sp.tile([128, G * 1], bf16, tag="er2")

            nc.vector.memset(h3[:], 0.0)
            tsh1 = dp.tile([NSH, 256], bf16)
            tfull1 = dp.tile([N, 256], bf16)
            tsh2 = dp.tile([NSH, 128], bf16)
            tfull2 = dp.tile([N, 128], bf16)

            def edge_stage(layer):
                if layer == 0:
                    table, rowc, nh, fdim = table0, 256, 2, 128
                    er_sb, res, bvec = er0, None, b0c
                elif layer == 1:
                    table, rowc, nh, fdim = tfull1, 256, 2, 128
                    er_sb, res, bvec = er1, h1_nd, b1c
                else:
                    table, rowc, nh, fdim = tfull2, 128, 1, OUT
                    er_sb, res, bvec = er2, None, b2c
                vsc = fdim + nh
                hd = fdim // nh
                idx_off = 0

                for pr in _pairs():
                    npg = len(pr)
                    g0 = pr[0]
                    nt = npg * tpg

                    st_sb = fp.tile([128, npg * tpg * 128], fp8, tag="st")
                    nc.sync.dma_start(
                        st_sb[:], st_in[:, g0:g0 + npg])
                    s_sb = fp.tile([128, npg * tpg * 128], fp8, tag="ss")
                    nc.sync.dma_start(
                        s_sb[:], ss_in[:, g0:g0 + npg])

                    v = fp.tile([128, nt, rowc], bf16, tag="v")
                    ncols = npg * cap // 16
                    for half in (0, 1):
                        ii = idx_sb[:, idx_off:idx_off + ncols]
                        idx_off += ncols
                        dst_v = v[:, half * (nt // 2):(half + 1) * (nt // 2), :]
                        src_t = table[0:HALF, :] if half == 0 else table[HALF:N, :]
                        nc.gpsimd.dma_gather(
                            dst_v, src_t, ii, npg * cap, npg * cap, rowc,
                            single_packet=False)

                    def vt(gi, t):
                        if t < th:
                            return gi * th + t
                        return npg * th + gi * th + (t - th)

                    # er_dst expand: one matmul per tile into striped PSUM
                    er_ps = pA.tile([128, nt * nh], f32, space="PSUM", tag="erp")
                    for gi in range(npg):
                        gg = pr[gi]
                        for t in range(tpg):
                            v_i = vt(gi, t)
                            nc.tensor.matmul(
                                out=er_ps[:, v_i * nh:(v_i + 1) * nh],
                                lhsT=s_sb[:, (gi * tpg + t) * 128:
                                          (gi * tpg + t + 1) * 128],
                                rhs=er_sb[:, gg * nh:(gg + 1) * nh],
                                start=True, stop=True)
                    score = mp.tile([128, nt * nh], f32, tag="score")
                    nc.vector.tensor_tensor(
                        out=score[:].rearrange("p (t h) -> p t h", h=nh),
                        in0=er_ps[:].rearrange("p (t h) -> p t h", h=nh),
                        in1=v[:, :, fdim:fdim + nh], op=_AL.add)
                    # p = exp(lrelu(s)) = max(exp(s), exp(NEG*s))
                    pa_t = mp.tile([128, nt * nh], f32, tag="pa")
                    pb_t = mp.tile([128, nt * nh], f32, tag="pb")
                    nc.scalar.activation(pa_t[:], score[:], _AF.Exp)
                    nc.scalar.activation(pb_t[:], score[:], _AF.Exp, scale=NEG)
                    p = mp.tile([128, nt * nh], bf16, tag="p")
                    nc.vector.tensor_max(p[:], pa_t[:], pb_t[:])

                    vs = fp.tile([128, nt, vsc], bf16, tag="vs")
                    nc.vector.tensor_tensor(
                        out=vs[:, :, 0:fdim].rearrange("p t (d h) -> p t d h", h=nh),
                        in0=v[:, :, 0:fdim].rearrange("p t (d h) -> p t d h", h=nh),
                        in1=p[:].rearrange("p (t h) -> p t h", h=nh)
                            .unsqueeze(2).to_broadcast([128, nt, hd, nh]),
                        op=_AL.mult)
                    nc.vector.tensor_copy(
                        vs[:, :, fdim:fdim + nh],
                        p[:].rearrange("p (t h) -> p t h", h=nh))

                    for gi in range(npg):
                        gg = pr[gi]
                        acc = pC.tile([128, vsc], f32, space="PSUM", tag="acc")
                        for t in range(tpg):
                            nc.tensor.matmul(
                                out=acc[:],
                                lhsT=st_sb[:, (gi * tpg + t) * 128:
                                           (gi * tpg + t + 1) * 128],
                                rhs=vs[:, vt(gi, t), :],
                                start=(t == 0), stop=(t == tpg - 1))
                        ssb = mp.tile([128, nh], f32, tag="ssb")
                        nc.vector.tensor_scalar(
                            ssb[:], acc[:, fdim:fdim + nh], 1e-30, None, _AL.max)
                        rs = mp.tile([128, nh], f32, tag="rs")
                        nc.vector.reciprocal(rs[:], ssb[:])
                        o = mp.tile([128, fdim], f32, tag="o")
                        ov = o[:].rearrange("p (d h) -> p d h", h=nh)
                        av = acc[:, 0:fdim].rearrange("p (d h) -> p d h", h=nh)
                        for h in range(nh):
                            nc.scalar.activation(
                                ov[:, :, h:h + 1], av[:, :, h:h + 1],
                                _AF.Copy, scale=rs[:, h:h + 1])
                        if layer == 2:
                            nc.vector.tensor_add(
                                h3[:, gg * OUT:(gg + 1) * OUT], o[:], b2c[:])
                            continue
                        xb = mp.tile([128, fdim], f32, tag="xb")
                        nc.vector.tensor_add(xb[:], o[:], bvec[:])
                        # elu(x) = max(x,0) + min(exp(min(x,0)) - 1, 0)
                        t1 = mp.tile([128, fdim], f32, tag="t1")
                        nc.vector.tensor_scalar_min(t1[:], xb[:], 0.0)
                        e1 = mp.tile([128, fdim], f32, tag="e1")
                        nc.scalar.activation(e1[:], t1[:], _AF.Exp)
                        t2 = mp.tile([128, fdim], f32, tag="t2")
                        nc.vector.tensor_scalar(
                            t2[:], e1[:], -1.0, 0.0, _AL.add, _AL.min)
                        t3 = mp.tile([128, fdim], f32, tag="t3")
                        nc.vector.tensor_scalar_max(t3[:], xb[:], 0.0)
                        elu = mp.tile([128, fdim], f32, tag="elu")
                        nc.vector.tensor_add(elu[:], t2[:], t3[:])
                        h_nd = h1_nd if layer == 0 else h2_nd
                        hsl = h_nd[:, gg * 128:(gg + 1) * 128]
                        if layer == 0:
                            xres = fp.tile([128, 128], bf16, tag="xres")
                            nc.sync.dma_start(
                                xres[:], x_nd[:, gg * 128:(gg + 1) * 128])
                            nc.vector.tensor_add(hsl, elu[:], xres[:])
                        else:
                            nc.vector.tensor_add(
                                hsl, elu[:], res[:, gg * 128:(gg + 1) * 128])
                        # next layer's node stage, fused
                        tp = pB.tile([128, 128], bf16, space="PSUM", tag="tp")
                        nc.tensor.transpose(out=tp[:], in_=hsl, identity=ident[:])
                        fsl = hfm[:, gg * 128:(gg + 1) * 128]
                        nc.scalar.copy(fsl, tp[:])
                        wn = w1 if layer == 0 else w2
                        wlen = wle1 if layer == 0 else wle2
                        fnext = 128 if layer == 0 else OUT
                        nhn = 2 if layer == 0 else 1
                        featp = pB.tile([128, fnext], f32, space="PSUM", tag="featp")
                        nc.tensor.matmul(out=featp[:], lhsT=fsl, rhs=wn[:],
                                         start=True, stop=True)
                        elp = pB.tile([128, 2 * nhn], f32, space="PSUM", tag="elp")
                        nc.tensor.matmul(out=elp[:], lhsT=fsl, rhs=wlen[:],
                                         start=True, stop=True)
                        rowcn = 256 if layer == 0 else 128
                        tt = mp.tile([128, rowcn], bf16, tag="ttile")
                        nc.vector.memset(tt[:, fnext + nhn:rowcn], 0)
                        nc.scalar.copy(tt[:, 0:fnext], featp[:])
                        nc.vector.tensor_copy(
                            tt[:, fnext:fnext + nhn], elp[:, 0:nhn])
                        ern = er1 if layer == 0 else er2
                        nc.vector.tensor_copy(
                            ern[:, gg * nhn:(gg + 1) * nhn], elp[:, nhn:2 * nhn])
                        tshn = tsh1 if layer == 0 else tsh2
                        nrows = min(128, NSH - gg * 128)
                        nc.sync.dma_start(
                            tshn[gg * 128:gg * 128 + nrows, :], tt[:nrows, :])

            def chunked_allgather(tsh, tfull):
                # per-chunk collectives overlap halo exchange with the
                # producing layer's tail groups; table rows are permuted on
                # the host (_new_row) so each chunk's output is contiguous
                bounds = _chunk_bounds()
                for lo, hi in zip(bounds[:-1], bounds[1:]):
                    nc.gpsimd.collective_compute(
                        "AllGather", _AL.bypass,
                        replica_groups=[list(range(NCORES))],
                        ins=[tsh[lo:hi, :].opt()],
                        outs=[tfull[NCORES * lo:NCORES * hi, :].opt()])

            edge_stage(0)
            if not skip_collectives:
                chunked_allgather(tsh1, tfull1)
            edge_stage(1)
            if not skip_collectives:
                chunked_allgather(tsh2, tfull2)
            edge_stage(2)

            # log_softmax over classes: x - ln(sum exp(x))
            ex = sp.tile([128, G * OUT], f32, tag="lsx")
            nc.scalar.activation(ex[:], h3[:], _AF.Exp)
            sm = sp.tile([128, G], f32, tag="lss")
            nc.vector.tensor_reduce(
                sm[:], ex[:].rearrange("p (g c) -> p g c", c=OUT),
                axis=mybir.AxisListType.X, op=_AL.add)
            ls = sp.tile([128, G], f32, tag="lsl")
            nc.scalar.activation(ls[:], sm[:], _AF.Ln)
            lsm = sp.tile([128, G * OUT], f32, tag="lsm")
            nc.vector.tensor_tensor(
                out=lsm[:].rearrange("p (g c) -> p g c", c=OUT),
                in0=h3[:].rearrange("p (g c) -> p g c", c=OUT),
                in1=ls[:].unsqueeze(2).to_broadcast([128, G, OUT]),
                op=_AL.subtract)
            nfull = NSH // 128
            nc.sync.dma_start(
                out_d[0:nfull * 128, :].rearrange("(g p) c -> p g c", p=128),
                lsm[:, 0:nfull * OUT].rearrange("p (g c) -> p g c", c=OUT))
            rem = NSH - nfull * 128
            if rem:
                nc.sync.dma_start(
                    out_d[nfull * 128:NSH, :],
                    lsm[0:rem, nfull * OUT:(nfull + 1) * OUT])

    nc.compile()
    return nc


_PROG_CACHE = {}


def kernel(x, src, dst, W0, al0, ar0, b0, W1, al1, ar1, b1,
           W2, al2, ar2, b2, trace=False):
    x = np.asarray(x, np.float32)
    src = np.asarray(src).astype(np.int64)
    dst = np.asarray(dst).astype(np.int64)
    W0, al0, ar0, b0 = (np.asarray(a, np.float32) for a in (W0, al0, ar0, b0))
    W1, al1, ar1, b1 = (np.asarray(a, np.float32) for a in (W1, al1, ar1, b1))
    W2, al2, ar2, b2 = (np.asarray(a, np.float32) for a in (W2, al2, ar2, b2))

    cores, cap, th, tpg = _preprocess(src, dst)

    # head-interleaved feature order: new col j=(d,h) <- orig col h*64+d.
    # Makes the per-edge attention scale broadcast innermost-stride-1 on DVE.
    PERM = np.array([(j % 2) * HID + j // 2 for j in range(128)])

    # host layer-0 node stage
    feat0 = (x @ W0).reshape(N, 2, HID)
    el0 = np.einsum("nhd,hd->nh", feat0, al0).astype(np.float32)
    er0 = np.einsum("nhd,hd->nh", feat0, ar0).astype(np.float32)
    table0 = np.zeros((N, 256), BF16)
    nr = _new_row()
    table0[nr, 0:128] = feat0.reshape(N, 128)[:, PERM].astype(BF16)
    table0[nr, 128:130] = el0.astype(BF16)

    W1p = W1[PERM, :]              # rows: h1 arrives interleaved
    wle1 = np.zeros((128, 4), np.float32)
    for h in range(2):
        wle1[:, h] = W1p[:, h * HID:(h + 1) * HID] @ al1[h]
        wle1[:, 2 + h] = W1p[:, h * HID:(h + 1) * HID] @ ar1[h]
    W1pi = W1p[:, PERM]            # cols: feat1 comes out interleaved
    W2p = W2[PERM, :]
    wle2 = np.zeros((128, 2), np.float32)
    wle2[:, 0] = W2p @ al2[0]
    wle2[:, 1] = W2p @ ar2[0]

    key = (cap,)
    if key not in _PROG_CACHE:
        _PROG_CACHE[key] = _build_program(cap, th, tpg)
    nc = _PROG_CACHE[key]

    in_maps = []
    for c in range(NCORES):
        cc = cores[c]
        in_maps.append(dict(
            table0=table0,
            x_nd=_node_major(x[:, PERM], c).astype(BF16),
            er0_in=_node_major(er0, c).astype(BF16),
            idx_in=cc["idx"],
            st_in=cc["st"],
            ss_in=cc["ss"],
            w1_in=W1pi.astype(BF16),
            wle1_in=wle1.astype(BF16),
            b0_in=np.tile(b0[None, PERM], (128, 1)).astype(np.float32),
            b1_in=np.tile(b1[None, PERM], (128, 1)).astype(np.float32),
            w2_in=W2p.astype(BF16),
            wle2_in=wle2.astype(BF16),
            b2_in=np.tile(b2[None, :], (128, 1)).astype(np.float32),
        ))
    res = run_bass_kernel_spmd(nc, in_maps, core_ids=list(range(NCORES)),
                               trace=trace)
    out = np.concatenate([res.results[c]["out_lsm"] for c in range(NCORES)],
                         axis=0).astype(np.float32)
    kernel._last_result = res
    return out
